# revision 45
# baseline (speedup 1.0000x reference)
"""Causal single-head attention (b=4, s=2048, d=1024) on 8 trn2 NeuronCores.

Sharding: data-parallel over batch (4) x 2-way key split per batch.
Core c = 2*b + h handles batch b and KEY tiles {2m+h : m=0..7} (128-row
tiles, interleaved so causal work stays balanced).

Key algebraic trick: scores = (X Wq)(X Wk)^T = X (Wq Wk^T) X^T, so the
host precomputes M = Wq Wk^T (exact, fp32) and the device computes only
A = X M ("merged Q") and contracts A against the raw X^T already
resident in SBUF. The entire K projection disappears. Each core:
  - computes V only for its own 8 key tiles (no duplication),
  - computes A^T for ALL 16 query tiles (duplicated across the pair),
  - runs a partial causal softmax over its key half for every query
    tile (scores = A . X^T_local / 32, the 1/32 applied as the exp
    activation scale), emitting the normalized partial output plus the
    row sums of exp-scores (no max-subtraction: scores are ~N(0,1)).
The host then merges the two partials per batch with an exact
weighted combine (weights = the two halves' exp-score row sums).

Causality per query tile t over local key tiles 0..t//2: the last local
tile is either the diagonal (triangular mask), fully visible, or fully
masked, depending only on parity(t) and the core's rank - handled by a
per-core additive mask tensor (data, not program), so the SPMD program
is identical across all 8 cores.

Scores are computed directly in TRANSPOSED layout, S^T[key, query], by
query-tile PAIRS (both tiles of pair j see the same j+1 local key
tiles): stationary = X^T key tile, moving = A^T pair columns (N=256).
exp(S^T) then lands in SBUF as P^T - immediately usable as the PV
stationary, eliminating all PE transposes and their DVE copies. Row
sums of exp-scores come from an N=1 ones-column matmul per key tile
that reuses the already-loaded PV stationary (~26ns each).

All matmuls run in fp16 (1 cyc/row on PE, fp32 PSUM accumulation);
softmax runs in fp32 on ACT/DVE. Pairs are software-pipelined: the
next pair's S^T tiles stream on the PE around PV(t0) so its softmax
and the O(t0) scale/store overlap matmuls; O(t1) is flushed at the
next pair's top.

Head/tail shaping (each dma_start costs ~650ns of serial sync-queue
dispatch, and the PE HAM clock-gate needs ~3.4us of sustained activity
to reach 2.4 GHz): V operands are DMA'd first (single_packet) and the
V build runs i-major in 2-tile groups so each arriving tile feeds the
PE immediately; the A projection is chunk-column-outer so the first two
score tiles + softmax overlap its remainder; the final slot stores in
quarter columns so scale + output DMA overlap the last matmuls.
"""

import sys
import types

import numpy as np

P = 128
SEQ = 2048
D = 1024
NB = 4
QT = SEQ // P      # 16 query tiles per core (all of them)
IT = D // P        # 8 contraction tiles (d_in)
OT = D // P        # 8 output tiles (d_out)
HT = QT // 2       # 8 key tiles per core
HCOL = HT * P      # 1024 local key columns
MASK_NEG = -30000.0
SCALE = 1.0 / 32.0  # 1/sqrt(d_out)

_PROG_CACHE = {}


def _install_ntff_hook():
    """Register the NTFF profile hook this image's antenv lacks (best effort)."""
    try:
        import antenv.axon_hooks  # noqa: F401
        return
    except ImportError:
        pass
    try:
        import trn_agent_boot.trn_boot as tb
        hook = tb._ntff_profile_via_ctypes('/opt/axon/libaxon_pjrt.so')
        mod = types.ModuleType('antenv.axon_hooks')
        mod._hook = hook
        mod.get_axon_ntff_profile_hook = lambda: mod._hook

        def _set(h):
            mod._hook = h
        mod.set_axon_ntff_profile_hook = _set
        sys.modules['antenv.axon_hooks'] = mod
    except Exception:
        pass


def build_program():
    """Build + compile the single SPMD Bass program (cached)."""
    if "nc" in _PROG_CACHE:
        return _PROG_CACHE["nc"]

    from contextlib import ExitStack

    import concourse.mybir as mybir
    from concourse import bacc
    from concourse.tile import TileContext

    f32 = mybir.dt.float32
    f16 = mybir.dt.float16
    ADD = mybir.AluOpType.add
    EXP = mybir.ActivationFunctionType.Exp

    nc = bacc.Bacc("TRN2", target_bir_lowering=False, debug=False, num_devices=8)

    # xk: the core's interleaved key-half columns of X^T (compacted);
    # xq: full X^T; wq: M = Wq Wk^T precomputed on host.
    xk_d = nc.dram_tensor("xk", [D, HCOL], f16, kind="ExternalInput").ap()
    xq_d = nc.dram_tensor("xq", [D, SEQ], f16, kind="ExternalInput").ap()
    wq_d = nc.dram_tensor("wq", [D, D], f16, kind="ExternalInput").ap()
    wv_d = nc.dram_tensor("wv", [D, D], f16, kind="ExternalInput").ap()
    mask_d = nc.dram_tensor("mask", [P, 2 * P], f32, kind="ExternalInput").ap()
    out_d = nc.dram_tensor("out", [SEQ, D], f16, kind="ExternalOutput").ap()
    ml_d = nc.dram_tensor("ml", [P, QT], f32, kind="ExternalOutput").ap()

    with TileContext(nc) as tc, ExitStack() as ctx:
        const = ctx.enter_context(tc.tile_pool(name="const", bufs=1))
        persist = ctx.enter_context(tc.tile_pool(name="persist", bufs=1))
        wpool = ctx.enter_context(tc.tile_pool(name="wpool", bufs=3))
        ptpool = ctx.enter_context(tc.tile_pool(name="ptpool", bufs=18))
        scal = ctx.enter_context(tc.tile_pool(name="scal", bufs=24))
        work = ctx.enter_context(tc.tile_pool(name="work", bufs=6, space="PSUM"))
        opsum = ctx.enter_context(tc.tile_pool(name="opsum", bufs=1, space="PSUM"))

        mask_sb = const.tile([P, 2 * P], f32, tag="mask")
        ones_sb = const.tile([P, 1], f16, tag="ones")
        nc.vector.memset(ones_sb[:], 1.0)

        # PE warm-up on a memset tile (no DMA dependency, so it starts the
        # instant the preamble ends): f32 matmuls bridge the DMA-dispatch-
        # limited head (each dma_start is ~650ns of serial sync-queue time)
        # so the HAM activity window fills early and the real matmul stream
        # runs at 2.4 GHz without stalling.
        junk_sb = const.tile([P, 256], f32, tag="junk")
        nc.vector.memset(junk_sb[:], 1.0)
        warm_ps = work.tile([P, 256], f32, tag="wk", name="warm_ps")
        for w in range(3):
            nc.tensor.matmul(
                warm_ps[:], lhsT=junk_sb[:, 0:P], rhs=junk_sb[:],
                start=(w == 0), stop=(w == 2),
            )

        # ---- input DMAs, ordered so V-build can start ASAP ----
        wv_sb = wpool.tile([P, IT, D], f16, tag="w", name="wv_sb")
        wq_sb = wpool.tile([P, IT, D], f16, tag="w", name="wq_sb")
        xk_sb = persist.tile([P, IT, HCOL], f16, tag="xk")
        xq_sb = persist.tile([P, IT, SEQ], f16, tag="xq")

        xk_t = xk_d.rearrange("(i p) s -> i p s", p=P)
        wv_t = wv_d.rearrange("(i p) o -> i p o", p=P)
        for i in range(IT):
            nc.sync.dma_start(out=wv_sb[:, i], in_=wv_t[i],
                              single_packet=True)
            nc.sync.dma_start(out=xk_sb[:, i], in_=xk_t[i],
                              single_packet=True)
        # the mask is not needed until the first score pair (~45us in), so
        # it queues behind all V operands instead of delaying them
        nc.sync.dma_start(out=mask_sb[:], in_=mask_d)
        wq_t = wq_d.rearrange("(i p) o -> i p o", p=P)
        xq_t = xq_d.rearrange("(i p) s -> i p s", p=P)
        for i in range(IT):
            nc.sync.dma_start(out=wq_sb[:, i], in_=wq_t[i],
                              single_packet=True)
            nc.sync.dma_start(out=xq_sb[:, i], in_=xq_t[i],
                              single_packet=True)

        stats_sb = persist.tile([P, QT], f32, tag="stats")
        nc.vector.memset(stats_sb[:], 0.0)
        qT_sb = persist.tile([P, OT, SEQ], f16, tag="qT")
        v_sb = persist.tile([P, HT, D], f16, tag="v")

        # ---- local V: v[m, o] = sum_i xk[i, m] * wv[i, o] ----
        # i-major in key-tile groups of 3/3/2: each arriving (wv[i], xk[i])
        # DMA pair immediately feeds 6 matmuls (~1.3us, matching the ~1.2us
        # 2-dispatch delivery cadence), so the PE paces the DMA trickle
        # without idle gaps and the HAM clock-gate warms early.
        for grp in ((0, 1, 2), (3, 4, 5), (6, 7)):
            vch = {st: [work.tile([P, 512], f32, tag="wk",
                                  name=f"vch{st}_{c}") for c in range(2)]
                   for st in grp}
            for i in range(IT):
                for st in grp:
                    lhsT = xk_sb[:, i, st * P:(st + 1) * P]
                    for c in range(2):
                        nc.tensor.matmul(
                            vch[st][c][:],
                            lhsT=lhsT,
                            rhs=wv_sb[:, i, c * 512:(c + 1) * 512],
                            start=(i == 0),
                            stop=(i == IT - 1),
                        )
            # drain the group's PSUM on both DVE and ACT so the next
            # group's pool slots free twice as fast (ACT is idle here)
            for sti, st in enumerate(grp):
                for c in range(2):
                    dst = v_sb[:, st, c * 512:(c + 1) * 512]
                    if (sti + c) % 2 == 0:
                        nc.vector.tensor_copy(out=dst, in_=vch[st][c][:])
                    else:
                        nc.scalar.activation(
                            dst, vch[st][c][:],
                            mybir.ActivationFunctionType.Copy,
                            bias=0.0, scale=1.0,
                        )

        def emit_stp(j, u0, u1, pts):
            """S^T tiles for query PAIR j (tiles 2j, 2j+1), local key tiles
            u0..u1-1. Each tile stp[u] = [128 keys, 256 queries] accumulates
            over the 8 contraction blocks, gets the parity mask (u == j:
            the diagonal/padded tile) added, and is exp'd to fp16 P^T in
            SBUF - directly usable as the PV stationary, no transposes."""
            for u in range(u0, u1):
                stp = work.tile([P, 256], f32, tag="wk", name=f"stp{j}_{u}")
                for o in range(OT):
                    nc.tensor.matmul(
                        stp[:],
                        lhsT=xk_sb[:, o, u * P:(u + 1) * P],
                        rhs=qT_sb[:, o, 2 * j * P:2 * j * P + 256],
                        start=(o == 0),
                        stop=(o == OT - 1),
                    )
                if u == j:
                    nc.vector.tensor_tensor(stp[:], stp[:], mask_sb[:], ADD)
                pt = ptpool.tile([P, 256], f16, tag="pt", name=f"pt{j}_{u}")
                nc.scalar.activation(
                    pt[:], stp[:], EXP, bias=0.0, scale=SCALE,
                )
                pts.append(pt)

        # ---- full A^T = M^T X^T (M = Wq Wk^T, so S = A X^T) ----
        # c-outer: the first 512 query columns finish first, letting the
        # first score pair compute + softmax while the rest of the A
        # projection still streams on the PE.
        stpq = []
        for c in range(4):
            for o in range(OT):
                ch = work.tile([P, 512], f32, tag="wk", name=f"qch{o}_{c}")
                for i in range(IT):
                    nc.tensor.matmul(
                        ch[:],
                        lhsT=wq_sb[:, i, o * P:(o + 1) * P],
                        rhs=xq_sb[:, i, c * 512:(c + 1) * 512],
                        start=(i == 0),
                        stop=(i == IT - 1),
                    )
                nc.vector.tensor_copy(
                    out=qT_sb[:, o, c * 512:(c + 1) * 512], in_=ch[:]
                )
            if c == 0:
                pair0 = []
                emit_stp(0, 0, 1, pair0)
                stpq.append(pair0)

        pending = []

        def flush_pending():
            o_ps_p, rinv_p, t_p = pending.pop()
            o_sb = wpool.tile([P, D], f16, tag="w", name=f"osb{t_p}")
            nc.scalar.mul(o_sb[:], o_ps_p[:], rinv_p[:])
            nc.sync.dma_start(out=out_d[t_p * P:(t_p + 1) * P, :], in_=o_sb[:])

        def emit_pv(t, pts, qoff, rs):
            """O(t) into the opsum slot + exp row sums, stationary shared
            between the two V chunks and the ones-column per key tile."""
            L = len(pts)
            o_ps = opsum.tile([P, D], f32, tag="o", name=f"ops{t}")
            for u in range(L):
                lhsT = pts[u][:, qoff:qoff + P]
                for c in range(2):
                    nc.tensor.matmul(
                        o_ps[:, c * 512:(c + 1) * 512],
                        lhsT=lhsT,
                        rhs=v_sb[:, u, c * 512:(c + 1) * 512],
                        start=(u == 0),
                        stop=(u == L - 1),
                    )
                nc.tensor.matmul(
                    rs[:], lhsT=lhsT, rhs=ones_sb[:],
                    start=(u == 0), stop=(u == L - 1),
                )
            nc.vector.tensor_copy(out=stats_sb[:, t:t + 1], in_=rs[:])
            rinv = scal.tile([P, 1], f32, tag="rinv", name=f"rinv{t}")
            nc.vector.reciprocal(rinv[:], rs[:])
            return o_ps, rinv

        # ---- software-pipelined attention over the 8 query-tile pairs ----
        for ji in range(HT):
            pts = stpq.pop(0)
            t0, t1 = 2 * ji, 2 * ji + 1
            L = ji + 1
            if pending:
                flush_pending()

            nxt = []
            half = (ji + 2 + 1) // 2
            if ji + 1 < HT:
                emit_stp(ji + 1, 0, half, nxt)

            def emit_pv_quarters(t, qoff, last, pts=pts, L=L):
                """Work-pool quarter-column PV + inline scale/store: used
                for the final pair so nothing waits on the opsum slot and
                the post-loop tail is a single quarter's scale + DMA."""
                rs = work.tile([P, 1], f32, tag="wk", name=f"rs{t}")
                o_sb = wpool.tile([P, D], f16, tag="w", name=f"osb{t}")
                rinv = None
                for c in range(4):
                    o_q = work.tile([P, 256], f32, tag="wk",
                                    name=f"opsq{t}_{c}")
                    for u in range(L):
                        nc.tensor.matmul(
                            o_q[:],
                            lhsT=pts[u][:, qoff:qoff + P],
                            rhs=v_sb[:, u, c * 256:(c + 1) * 256],
                            start=(u == 0),
                            stop=(u == L - 1),
                        )
                        if c == 0:
                            nc.tensor.matmul(
                                rs[:], lhsT=pts[u][:, qoff:qoff + P],
                                rhs=ones_sb[:],
                                start=(u == 0), stop=(u == L - 1),
                            )
                    if c == 0:
                        nc.vector.tensor_copy(
                            out=stats_sb[:, t:t + 1], in_=rs[:]
                        )
                        rinv = scal.tile([P, 1], f32, tag="rinv",
                                         name=f"rinv{t}")
                        nc.vector.reciprocal(rinv[:], rs[:])
                        if last:
                            nc.sync.dma_start(out=ml_d[:], in_=stats_sb[:])
                    qtr = slice(c * 256, (c + 1) * 256)
                    nc.scalar.mul(o_sb[:, qtr], o_q[:], rinv[:])
                    # the very last store goes out on the scalar HWDGE
                    # queue: no cross-engine semaphore hop after the final
                    # scale, shortening the kernel tail
                    eng = nc.scalar if (last and c == 3) else nc.sync
                    eng.dma_start(
                        out=out_d[t * P:(t + 1) * P, qtr], in_=o_sb[:, qtr]
                    )

            if ji < HT - 1:
                rs0 = work.tile([P, 1], f32, tag="wk", name=f"rs{t0}")
                o_ps0, rinv0 = emit_pv(t0, pts, 0, rs0)
                # flush t0 immediately: the ACT scale runs while the PE
                # streams the second half of the next pair's scores,
                # freeing the opsum slot before PV(t1) needs it
                o_sb0 = wpool.tile([P, D], f16, tag="w", name=f"osb{t0}")
                nc.scalar.mul(o_sb0[:], o_ps0[:], rinv0[:])
                nc.sync.dma_start(
                    out=out_d[t0 * P:(t0 + 1) * P, :], in_=o_sb0[:]
                )
                emit_stp(ji + 1, half, ji + 2, nxt)
                stpq.append(nxt)
                rs1 = work.tile([P, 1], f32, tag="wk", name=f"rs{t1}")
                o_ps1, rinv1 = emit_pv(t1, pts, P, rs1)
                pending.append((o_ps1, rinv1, t1))
            else:
                emit_pv_quarters(t0, 0, last=False)
                emit_pv_quarters(t1, P, last=True)

        if pending:
            flush_pending()

    nc.compile()
    _PROG_CACHE["nc"] = nc
    return nc


def make_in_maps(x, Wq, Wk, Wv):
    """Host-side sharding: returns per-core input dicts (core c = 2*b + h)."""
    x = np.asarray(x, dtype=np.float32)
    # scores = (X Wq)(X Wk)^T = X (Wq Wk^T) X^T: fold Wq,Wk into one
    # matrix M (exact fp32 on host); the kernel's K projection vanishes.
    mqk = np.asarray(Wq, dtype=np.float32) @ np.asarray(Wk, dtype=np.float32).T
    wq16 = mqk.astype(np.float16)
    wv16 = np.asarray(Wv, dtype=np.float32).astype(np.float16)

    # S^T-layout additive mask for local key tile u == j of query pair j
    # (tiles t0=2j, t1=2j+1); the tile's absolute key tile is k = 2j + h:
    #   h=0: k==t0 -> diagonal (triT) for t0, fully visible for t1
    #   h=1: k>t0 -> fully masked for t0, diagonal (triT) for t1
    # triT[key r, query c]: visible iff c >= r.
    tri_t = np.where(
        np.arange(P)[None, :] >= np.arange(P)[:, None], 0.0, MASK_NEG
    ).astype(np.float32)
    full = np.full((P, P), MASK_NEG, dtype=np.float32)
    zero = np.zeros((P, P), dtype=np.float32)
    masks = [
        np.concatenate([tri_t, zero], axis=1),   # h = 0
        np.concatenate([full, tri_t], axis=1),   # h = 1
    ]

    in_maps = []
    for b in range(NB):
        xt16 = np.ascontiguousarray(x[b].T).astype(np.float16)  # [D, SEQ]
        for h in range(2):
            kcols = np.concatenate(
                [np.arange((2 * m + h) * P, (2 * m + h + 1) * P)
                 for m in range(HT)]
            )
            in_maps.append({
                "xk": np.ascontiguousarray(xt16[:, kcols]),
                "xq": xt16,
                "wq": wq16,
                "wv": wv16,
                "mask": masks[h],
            })
    return in_maps


def assemble_output(results):
    """Log-sum-exp combine of the two partial softmax halves per batch."""
    out = np.empty((NB, SEQ, D), dtype=np.float32)
    for b in range(NB):
        r0 = results[2 * b]
        r1 = results[2 * b + 1]
        o0 = r0["out"].astype(np.float64)
        o1 = r1["out"].astype(np.float64)
        # ml is [P, QT]: col t = row sums of exp-scores; q = t*128+p
        def lsum(r):
            m = r["ml"].astype(np.float64)
            return m.T.reshape(SEQ)
        w0 = lsum(r0)
        w1 = lsum(r1)
        tot = w0 + w1
        w0 /= tot
        w1 /= tot
        # a zero-weight half may carry inf/nan partials (1/l with l=0)
        acc = np.where(w0[:, None] > 0, o0 * w0[:, None], 0.0)
        acc += np.where(w1[:, None] > 0, o1 * w1[:, None], 0.0)
        out[b] = acc.astype(np.float32)
    return out


def run(inputs, trace=False, tmpdir=None):
    """Build, run on 8 cores, gather. Returns (output, BassKernelResults)."""
    _install_ntff_hook()
    from concourse.bass_utils import run_bass_kernel_spmd

    nc = build_program()
    in_maps = make_in_maps(
        inputs["x"], inputs["Wq"], inputs["Wk"], inputs["Wv"]
    )
    kw = {}
    if trace:
        kw["trace"] = True
        if tmpdir is not None:
            kw["tmpdir"] = tmpdir
    res = run_bass_kernel_spmd(nc, in_maps, list(range(8)), **kw)
    return assemble_output(res.results), res


def kernel(**inputs):
    out, _ = run(inputs, trace=False)
    return out

